# revision 1
# baseline (speedup 1.0000x reference)
"""Dense MoE (BasicMoE) Trainium2 Bass kernel.

Problem (hardcoded): x [4, 2048, 1024] f32, gate_w [1024, 8], gate_b [8],
expert_w [8, 1024, 1024], expert_b [8, 1024].

    tok = x.reshape(T, H)
    w   = softmax(tok @ gate_w + gate_b)           # [T, E]
    eo  = einsum('th,ehd->ted', tok, expert_w) + expert_b
    out = einsum('te,ted->td', w, eo)              # [T, H]

Sharding: tokens split across 8 cores (data parallel), weights replicated.

Per-core algorithm (T_l = 1024 tokens). The TensorEngine contracts along
the partition dim, so the activations are needed h-major (xT); that
transpose is pure data layout, done host-side when sharding.

  1. Gate, in transposed [e, t] layout: logitsT = gate_w.T @ x.T via
     matmuls with gate_w slices stationary (8-wide LDWEIGHTS);
     ewT = exp(logitsT + gate_b) with gate_b as a natural per-partition
     ACT bias. Small PE transposes give ew in [t, e] layout, where
     S = sum_e ew and ews = ew * (1/S) are per-partition ops. softmax's
     division is linear in the combine, so it is folded into the gate
     weights and nothing needs normalizing at the end. ews is transposed
     back (ewsT) only when a bias term is needed.
  2. When expert_b is nonzero, acc[t,d] is seeded with the bias term
     sum_e ews[t,e]*b_e[d] (K=8 matmul of ewsT against expert_b); the
     build is specialized per np.any(expert_b) and cached per variant —
     for all-zero bias, expert 0 writes acc directly.
  3. For each expert: y_e = xT.T @ W_e accumulated over k in PSUM
     (bf16 operands, fp32 accumulation, full PE rate with fast weight
     load), evicted scaled by ews[:,e] (per-partition scale, alternating
     ACT/DVE) and added into an SBUF accumulator by DVE.
  4. acc IS the output: DMA out per 512-wide half as soon as the last
     expert's contribution lands.
"""

import os
from contextlib import ExitStack

import numpy as np

import concourse.tile as tile
from concourse import bacc, mybir
from concourse.bass_utils import run_bass_kernel_spmd
from concourse.masks import make_identity

B, S, H, E = 4, 2048, 1024, 8
T = B * S
N_CORES = 8
TL = T // N_CORES          # tokens per core = 1024
P = 128                    # SBUF partitions
KT = H // P                # 8 contraction tiles
MT = TL // P               # 8 token tiles per core
DH = 512                   # matmul moving free-dim (fp32 PSUM bank)
ND = H // DH               # 2 d-halves
XC = 2                     # x DMA column chunks (queue parallelism)
OC = 2                     # output DMA column chunks per (m, half)

F32 = mybir.dt.float32
F32R = mybir.dt.float32r
BF16 = mybir.dt.bfloat16

_CACHE = {}
LAST_RESULT = None


def _r(ap):
    """Bitcast an f32 AP to float32r (same bits; PE rounds internally)."""
    return ap.bitcast(F32R)


def _build_moe_nc(with_bias: bool):
    nc = bacc.Bacc(
        "TRN2",
        target_bir_lowering=False,
        debug=False,
        enable_asserts=False,
        num_devices=N_CORES,
    )

    x_shT = nc.dram_tensor("x_shT", [H, TL], BF16, kind="ExternalInput").ap()
    gate_w = nc.dram_tensor("gate_w", [H, E], BF16, kind="ExternalInput").ap()
    gate_b = nc.dram_tensor("gate_b", [E], F32, kind="ExternalInput").ap()
    expert_w = nc.dram_tensor("expert_w", [E, H, H], BF16, kind="ExternalInput").ap()
    expert_b = nc.dram_tensor("expert_b", [E, H], F32, kind="ExternalInput").ap()
    out_sh = nc.dram_tensor("out_sh", [TL, H], F32, kind="ExternalOutput").ap()

    with tile.TileContext(nc) as tc, ExitStack() as ctx:
        const = ctx.enter_context(tc.tile_pool(name="const", bufs=1))
        wpool = ctx.enter_context(tc.tile_pool(name="wpool", bufs=2))
        accp = ctx.enter_context(tc.tile_pool(name="accp", bufs=1))
        tmp = ctx.enter_context(tc.tile_pool(name="tmp", bufs=6))
        # main psum pool FIRST: its banks must not overlap the gate pool's,
        # else Tile's released-zone dep would stall expert 0's first matmul
        # group behind the whole gate phase.
        psum = ctx.enter_context(tc.tile_pool(name="psum", bufs=6, space="PSUM"))
        psum_s = tc.alloc_tile_pool(name="psum_s", bufs=1, space="PSUM")

        ident = const.tile([P, P], F32)
        make_identity(nc, ident)

        ident_bf = const.tile([E, E], BF16)
        make_identity(nc, ident_bf)

        # ---- loads ------------------------------------------------------
        gw = const.tile([P, KT, E], BF16)
        for k in range(KT):
            nc.sync.dma_start(gw[:, k, :], gate_w[k * P : (k + 1) * P, :])
        gb8 = const.tile([E, 1], F32)
        nc.sync.dma_start(gb8, gate_b[:, None])
        if with_bias:
            eb = const.tile([E, H], F32R)
            nc.sync.dma_start(eb, _r(expert_b))

        # xT: h on partitions, t on free — straight (contiguous) DMA from the
        # host-transposed shard. GpSimd SWDGE queues, half-column chunks in
        # half-major order so the first gate matmul's operands land first.
        xT = [const.tile([P, TL], BF16, name=f"xT{k}") for k in range(KT)]
        xcw = TL // XC
        for c in range(XC):
            for k in range(KT):
                csl = slice(c * xcw, (c + 1) * xcw)
                nc.gpsimd.dma_start(
                    xT[k][:, csl], x_shT[k * P : (k + 1) * P, csl]
                )

        # ---- gate -------------------------------------------------------
        ewT_raw = const.tile([E, TL], BF16)   # exp(logits).T (unnormalized)
        ews = const.tile([P, MT, E], F32)     # per-token gate weight / S
        ewsT = None
        if with_bias:
            ewsT = const.tile([E, TL], F32R, name="ewsT")

        for h2 in range(2):
            hsl = slice(h2 * DH, (h2 + 1) * DH)
            pgT = psum_s.tile([E, DH], F32, tag="sm", bufs=2)
            for k in range(KT):
                nc.tensor.matmul(
                    pgT,
                    lhsT=gw[:, k, :],
                    rhs=xT[k][:, hsl],
                    start=(k == 0),
                    stop=(k == KT - 1),
                )
            # ewT = exp(logitsT + gate_b); gate_b is per-partition here
            nc.scalar.activation(
                ewT_raw[:, hsl], pgT, mybir.ActivationFunctionType.Exp, bias=gb8
            )

        for m in range(MT):
            msl = slice(m * P, (m + 1) * P)
            # ew[t, e] for this token tile via PE transpose
            ptw = psum_s.tile([P, E], BF16, tag="sm", bufs=2)
            nc.tensor.transpose(ptw, ewT_raw[:, msl], ident_bf)
            ssum = tmp.tile([P, 1], F32, tag="ssum")
            nc.vector.reduce_sum(ssum, ptw, axis=mybir.AxisListType.X)
            inv = tmp.tile([P, 1], F32, tag="inv")
            nc.vector.reciprocal(inv, ssum)
            nc.vector.tensor_scalar_mul(ews[:, m, :], ptw, inv)
            if with_bias:
                # back-transpose the normalized weights for the bias matmul
                ptb = psum_s.tile([E, P], F32, tag="sm", bufs=2)
                nc.tensor.transpose(ptb, ews[:, m, :], ident)
                nc.vector.tensor_copy(ewsT[:, msl], _r(ptb))

        # gate done; its two banks are no longer needed
        psum_s.release()

        # ---- bias seed: acc = ews @ expert_b (skipped for zero bias) ---
        acc = [accp.tile([P, H], F32, name=f"acc{m}") for m in range(MT)]
        if with_bias:
            for m in range(MT):
                msl = slice(m * P, (m + 1) * P)
                for n in range(ND):
                    nsl = slice(n * DH, (n + 1) * DH)
                    pb = psum.tile([P, DH], F32, tag="ps")
                    nc.tensor.matmul(
                        pb, lhsT=ewsT[:, msl], rhs=eb[:, nsl], start=True, stop=True
                    )
                    nc.vector.tensor_copy(acc[m][:, nsl], pb)

        # ---- experts ----------------------------------------------------
        ocw = DH // OC
        for e in range(E):
            wsb = wpool.tile([P, KT, H], BF16, tag="w")
            # e<2 are latency-critical (PE is waiting): small chunks split
            # across both HWDGE and SWDGE queue sets. Steady state uses one
            # sync chunk per k.
            ewc = 4 if e == 0 else (2 if e == 1 else 1)
            wcw = H // ewc
            for c in range(ewc):
                for k in range(KT):
                    csl = slice(c * wcw, (c + 1) * wcw)
                    eng = nc.gpsimd if (e < 2 and k % 2 == 1) else nc.sync
                    eng.dma_start(
                        wsb[:, k, csl],
                        expert_w[e, k * P : (k + 1) * P, csl],
                    )
            last = e == E - 1
            for n in range(ND):
                nsl = slice(n * DH, (n + 1) * DH)
                for m in range(MT):
                    msl = slice(m * P, (m + 1) * P)
                    ps = psum.tile([P, DH], F32, tag="ps")
                    for k in range(KT):
                        nc.tensor.matmul(
                            ps,
                            lhsT=xT[k][:, msl],
                            rhs=wsb[:, k, nsl],
                            start=(k == 0),
                            stop=(k == KT - 1),
                        )
                    # evict scaled by normalized gate weight; alternate the
                    # scale between ACT and DVE so neither engine saturates.
                    # With no bias seed, expert 0 writes acc directly.
                    if e == 0 and not with_bias:
                        if (m + n) % 2 == 0:
                            nc.scalar.mul(acc[m][:, nsl], ps, ews[:, m, e : e + 1])
                        else:
                            nc.vector.tensor_scalar_mul(
                                acc[m][:, nsl], ps, ews[:, m, e : e + 1]
                            )
                    else:
                        t = tmp.tile([P, DH], F32, tag="evict")
                        if (m + n) % 2 == 0:
                            nc.scalar.mul(t, ps, ews[:, m, e : e + 1])
                        else:
                            nc.vector.tensor_scalar_mul(t, ps, ews[:, m, e : e + 1])
                        nc.vector.tensor_add(acc[m][:, nsl], acc[m][:, nsl], t)
                    if last:
                        noc = OC * 2 if m == MT - 1 else OC
                        for c in range(noc):
                            ocw2 = DH // noc
                            osl = slice(
                                n * DH + c * ocw2, n * DH + (c + 1) * ocw2
                            )
                            nc.sync.dma_start(
                                out_sh[m * P : (m + 1) * P, osl],
                                acc[m][:, osl],
                            )

    nc.compile()
    return nc


def kernel(**inputs) -> np.ndarray:
    global LAST_RESULT
    import ml_dtypes

    bf16 = ml_dtypes.bfloat16
    x = np.asarray(inputs["x"], dtype=np.float32).reshape(T, H)
    gw = np.ascontiguousarray(np.asarray(inputs["gate_w"], dtype=np.float32).astype(bf16))
    gb = np.ascontiguousarray(np.asarray(inputs["gate_b"], dtype=np.float32))
    ew = np.ascontiguousarray(np.asarray(inputs["expert_w"], dtype=np.float32).astype(bf16))
    eb = np.ascontiguousarray(np.asarray(inputs["expert_b"], dtype=np.float32))

    with_bias = bool(np.any(eb))
    key = ("nc", with_bias)
    if key not in _CACHE:
        _CACHE[key] = _build_moe_nc(with_bias)
    nc = _CACHE[key]

    in_maps = [
        {
            "x_shT": np.ascontiguousarray(x[c * TL : (c + 1) * TL].T.astype(bf16)),
            "gate_w": gw,
            "gate_b": gb,
            "expert_w": ew,
            "expert_b": eb,
        }
        for c in range(N_CORES)
    ]
    res = run_bass_kernel_spmd(
        nc,
        in_maps,
        core_ids=list(range(N_CORES)),
        trace=bool(int(os.environ.get("MOE_TRACE", "0"))),
    )
    LAST_RESULT = res
    out = np.concatenate([res.results[c]["out_sh"] for c in range(N_CORES)], axis=0)
    return out.reshape(B, S, H)



# revision 2
# speedup vs baseline: 1.1932x; 1.1932x over previous
"""Dense MoE (BasicMoE) Trainium2 Bass kernel — v2.

Problem (hardcoded): x [4, 2048, 1024] f32, gate_w [1024, 8], gate_b [8],
expert_w [8, 1024, 1024], expert_b [8, 1024].

    tok = x.reshape(T, H)
    w   = softmax(tok @ gate_w + gate_b)           # [T, E]
    eo  = einsum('th,ehd->ted', tok, expert_w) + expert_b
    out = einsum('te,ted->td', w, eo)              # [T, H]

Sharding: tokens split across 8 cores (data parallel), weights replicated.

v2 changes vs the baseline (256.5us):
  * Gate computed directly in [t, e] layout (lhsT = xT token tile, rhs =
    gate_w k-slice) — no PE transposes, softmax is 3 tiny DVE ops per
    token tile in natural per-partition layout.
  * Packed gate weights ride as 8 extra columns of the host-side x
    shard tensor ([H, TL+8]) — they arrive with the xT DMAs at zero
    extra descriptor cost and the gate needs no separate weight load.
  * DMA issue is descriptor-rate bound (~5ns/descriptor on the issuing
    engine): xT is split across the scalar+gpsimd queue families with
    full-width 2KB-row chunks, expert weights stream full-width k-chunks
    on sync. This gets the critical first 4MB (xT + W0) onto the wire at
    max rate, so the PE goes dense at ~8.5us instead of ~22us.
  * 3 dummy warmup matmuls on a zeroed tile right after the framework
    barrier: they cover the first-DMA latency and start the HAM clock
    warmup window ~2us earlier (PE at 2.4GHz by the first k rounds).
  * PSUM: 6 expert banks + 2 small gate banks. Expert accumulation
    groups self-organize into k-waves behind the arriving chunks.
  * Combine fused: acc = psum*w + acc in ONE DVE scalar_tensor_tensor
    per group (was scale + add), alternating with an ACT-scale +
    GpSimd-add pair so no single engine saturates.
  * Last expert evicts all-DVE with the output DMA per (n, m) chunk
    issued immediately, alternating sync/scalar queues; final chunk is
    split across both queues to shorten the tail.
"""

import os
from contextlib import ExitStack

import numpy as np

import concourse.tile as tile
from concourse import bacc, mybir
from concourse.bass_utils import run_bass_kernel_spmd

B, S, H, E = 4, 2048, 1024, 8
T = B * S
N_CORES = 8
TL = T // N_CORES          # tokens per core = 1024
TLG = TL + E               # + packed gate-weight columns
P = 128                    # SBUF partitions
KT = H // P                # 8 contraction tiles
MT = TL // P               # 8 token tiles per core
DH = 512                   # matmul moving free-dim (fp32 PSUM bank)
ND = H // DH               # 2 d-halves

F32 = mybir.dt.float32
BF16 = mybir.dt.bfloat16
MULT = mybir.AluOpType.mult
ADD = mybir.AluOpType.add

_CACHE = {}
LAST_RESULT = None


def _build_moe_nc(with_gb: bool, with_eb: bool):
    nc = bacc.Bacc(
        "TRN2",
        target_bir_lowering=False,
        debug=False,
        enable_asserts=False,
        num_devices=N_CORES,
    )

    # x shard transposed with gate_w packed into the last E columns:
    # x_shT[h, :TL] = x.T, x_shT[h, TL+e] = gate_w[h, e]
    x_shT = nc.dram_tensor("x_shT", [H, TLG], BF16, kind="ExternalInput").ap()
    expert_w = nc.dram_tensor("expert_w", [E, H, H], BF16, kind="ExternalInput").ap()
    # scratch target for the W1-throttle DMA (see below)
    scr = nc.dram_tensor("scr", [1, 8], F32, kind="Internal").ap()
    if with_gb:
        gate_b = nc.dram_tensor("gate_b", [E], F32, kind="ExternalInput").ap()
    if with_eb:
        expert_b = nc.dram_tensor("expert_b", [E, H], F32, kind="ExternalInput").ap()
    out_sh = nc.dram_tensor("out_sh", [TL, H], F32, kind="ExternalOutput").ap()

    with tile.TileContext(nc) as tc, ExitStack() as ctx:
        psum = ctx.enter_context(tc.tile_pool(name="psum", bufs=6, space="PSUM"))
        const = ctx.enter_context(tc.tile_pool(name="const", bufs=1))
        wpool = ctx.enter_context(tc.tile_pool(name="wpool", bufs=2))
        accp = ctx.enter_context(tc.tile_pool(name="accp", bufs=1))
        tmp = ctx.enter_context(tc.tile_pool(name="tmp", bufs=4))

        # ---- tiles ------------------------------------------------------
        dummy = const.tile([P, DH], BF16)
        nc.vector.memset(dummy, 0.0)

        xT = [const.tile([P, TLG], BF16, name=f"xT{k}") for k in range(KT)]
        ews = const.tile([P, MT, E], F32)
        acc = [accp.tile([P, H], F32, name=f"acc{m}") for m in range(MT)]

        # ---- DMA issue --------------------------------------------------
        # Each DMA queue family (sync/scalar/gpsimd) tops out around
        # ~150GB/s, well under the ~358GB/s per-core HBM share, so the
        # startup-critical stream must be striped across all three.
        # Pair (xT[k], W0-n0[k]) rides family k%3 so each expert-0
        # k-round's operands arrive together, in k order per family.
        wsb0 = wpool.tile([P, KT, H], BF16, tag="w", name="wsb0")
        fams = [nc.sync, nc.scalar, nc.gpsimd]
        for k in range(KT):
            eng = fams[k % 3]
            eng.dma_start(xT[k], x_shT[k * P : (k + 1) * P, :])
            eng.dma_start(
                wsb0[:, k, 0:DH], expert_w[0, k * P : (k + 1) * P, 0:DH]
            )
        # W0-n1 halves follow behind the critical stream on each family.
        for k in range(KT):
            fams[k % 3].dma_start(
                wsb0[:, k, DH:H], expert_w[0, k * P : (k + 1) * P, DH:H]
            )

        if with_gb:
            gb_sb = const.tile([1, E], F32)
            nc.gpsimd.dma_start(gb_sb, gate_b[None, :])
            ones_col = const.tile([1, P], F32)
            nc.vector.memset(ones_col, 1.0)
        if with_eb:
            eb_sb = const.tile([E, H], F32)
            nc.gpsimd.dma_start(eb_sb, expert_b)

        # ---- PE warmup: dummy matmuls (no DMA deps) ---------------------
        # Bridge the ~3us from the framework barrier to the first data
        # arrival and start the HAM clock-warmup window early.
        for _ in range(5):
            psd = psum.tile([P, DH], F32, tag="ps")
            nc.tensor.matmul(psd, lhsT=dummy[:, :P], rhs=dummy, start=True, stop=True)

        # ---- gate: logits in [t, e] layout, no transposes ---------------
        if with_gb:
            # broadcast gate_b across partitions via a K=1 outer product
            pgb = psum.tile([P, E], F32, tag="pg", bufs=2)
            nc.tensor.matmul(pgb, lhsT=ones_col, rhs=gb_sb, start=True, stop=True)
            gb_bc = const.tile([P, E], F32)
            nc.vector.tensor_copy(gb_bc, pgb)

        for m in range(MT):
            msl = slice(m * P, (m + 1) * P)
            pg = psum.tile([P, E], F32, tag="pg", bufs=2)
            for k in range(KT):
                nc.tensor.matmul(
                    pg,
                    lhsT=xT[k][:, msl],
                    rhs=xT[k][:, TL:TLG],
                    start=(k == 0),
                    stop=(k == KT - 1),
                )
            ew_raw = tmp.tile([P, E], F32, tag="ewr")
            if with_gb:
                logadd = tmp.tile([P, E], F32, tag="ladd")
                nc.vector.tensor_add(logadd, pg, gb_bc)
                nc.scalar.activation(
                    ew_raw, logadd, mybir.ActivationFunctionType.Exp
                )
            else:
                nc.scalar.activation(ew_raw, pg, mybir.ActivationFunctionType.Exp)
            ssum = tmp.tile([P, 1], F32, tag="ssum")
            nc.vector.reduce_sum(ssum, ew_raw, axis=mybir.AxisListType.X)
            inv = tmp.tile([P, 1], F32, tag="inv")
            nc.vector.reciprocal(inv, ssum)
            nc.vector.tensor_scalar_mul(ews[:, m, :], ew_raw, inv)

        # ---- optional bias seed: acc = ews @ expert_b -------------------
        if with_eb:
            from concourse.masks import make_identity

            ident = const.tile([P, P], F32)
            make_identity(nc, ident)
            for m in range(MT):
                ptb = psum.tile([E, P], F32, tag="pg", bufs=2)
                nc.tensor.transpose(ptb, ews[:, m, :], ident)
                ewsT_m = tmp.tile([E, P], F32, tag="ewsT")
                nc.vector.tensor_copy(ewsT_m, ptb)
                for n in range(ND):
                    nsl = slice(n * DH, (n + 1) * DH)
                    pb = psum.tile([P, DH], F32, tag="ps")
                    nc.tensor.matmul(
                        pb, lhsT=ewsT_m, rhs=eb_sb[:, nsl], start=True, stop=True
                    )
                    nc.vector.tensor_copy(acc[m][:, nsl], pb)

        # ---- experts ----------------------------------------------------
        wsb = wsb0
        for e in range(E):
            if e > 0:
                if e == 1:
                    # Throttle: hold W1 (and everything behind it on the
                    # sync queue) off the HBM wire until expert 0's first
                    # evict lands, so the startup-critical xT + W0 stream
                    # gets the full bandwidth. The tiny SBUF->DRAM DMA
                    # blocks the sync engine on the evict's semaphore.
                    nc.sync.dma_start(scr, acc[0][0:1, 0:8])
                wsb = wpool.tile([P, KT, H], BF16, tag="w", name=f"wsb{e}")
                for k in range(KT):
                    nc.sync.dma_start(
                        wsb[:, k, :], expert_w[e, k * P : (k + 1) * P, :]
                    )
            last = e == E - 1
            for n in range(ND):
                nsl = slice(n * DH, (n + 1) * DH)
                for m in range(MT):
                    msl = slice(m * P, (m + 1) * P)
                    wsc = ews[:, m, e : e + 1]
                    if last and m == MT - 1 and n == ND - 1:
                        # final group runs as two 256-wide groups so the
                        # first half's evict+DMA overlap the second
                        # half's matmuls, shortening the kernel tail.
                        hw = DH // 2
                        for h2 in range(2):
                            csl = slice(n * DH + h2 * hw, n * DH + (h2 + 1) * hw)
                            psh = psum.tile([P, hw], F32, tag="ps", name="psh")
                            for k in range(KT):
                                nc.tensor.matmul(
                                    psh,
                                    lhsT=xT[k][:, msl],
                                    rhs=wsb[:, k, csl],
                                    start=(k == 0),
                                    stop=(k == KT - 1),
                                )
                            nc.vector.scalar_tensor_tensor(
                                acc[m][:, csl], psh, wsc, acc[m][:, csl],
                                MULT, ADD,
                            )
                            eng = nc.sync if h2 == 0 else nc.scalar
                            eng.dma_start(out_sh[msl, csl], acc[m][:, csl])
                        continue
                    ps = psum.tile([P, DH], F32, tag="ps")
                    for k in range(KT):
                        nc.tensor.matmul(
                            ps,
                            lhsT=xT[k][:, msl],
                            rhs=wsb[:, k, nsl],
                            start=(k == 0),
                            stop=(k == KT - 1),
                        )
                    if e == 0 and not with_eb:
                        # first expert writes acc directly (scale only)
                        if (m + n) % 2 == 0:
                            nc.scalar.mul(acc[m][:, nsl], ps, wsc)
                        else:
                            nc.vector.tensor_scalar_mul(acc[m][:, nsl], ps, wsc)
                    elif last or (m + n) % 2 == 0:
                        # fused acc = ps*w + acc on DVE
                        nc.vector.scalar_tensor_tensor(
                            acc[m][:, nsl], ps, wsc, acc[m][:, nsl], MULT, ADD
                        )
                    else:
                        # spread load: ACT scale + GpSimd add (SBUF only)
                        t = tmp.tile([P, DH], F32, tag="ev")
                        nc.scalar.mul(t, ps, wsc)
                        nc.gpsimd.tensor_add(acc[m][:, nsl], acc[m][:, nsl], t)
                    if last:
                        eng = nc.sync if (m + n) % 2 == 0 else nc.scalar
                        eng.dma_start(out_sh[msl, nsl], acc[m][:, nsl])

    nc.compile()
    return nc


def kernel(**inputs) -> np.ndarray:
    global LAST_RESULT
    import ml_dtypes

    bf16 = ml_dtypes.bfloat16
    x = np.asarray(inputs["x"], dtype=np.float32).reshape(T, H)
    gw = np.asarray(inputs["gate_w"], dtype=np.float32).astype(bf16)
    gb = np.ascontiguousarray(np.asarray(inputs["gate_b"], dtype=np.float32))
    ew = np.ascontiguousarray(np.asarray(inputs["expert_w"], dtype=np.float32).astype(bf16))
    eb = np.ascontiguousarray(np.asarray(inputs["expert_b"], dtype=np.float32))

    with_gb = bool(np.any(gb))
    with_eb = bool(np.any(eb))
    key = ("nc", with_gb, with_eb)
    if key not in _CACHE:
        _CACHE[key] = _build_moe_nc(with_gb, with_eb)
    nc = _CACHE[key]

    in_maps = []
    for c in range(N_CORES):
        xa = np.empty((H, TLG), dtype=bf16)
        xa[:, :TL] = x[c * TL : (c + 1) * TL].T.astype(bf16)
        xa[:, TL:] = gw
        m = {"x_shT": xa, "expert_w": ew}
        if with_gb:
            m["gate_b"] = gb
        if with_eb:
            m["expert_b"] = eb
        in_maps.append(m)

    res = run_bass_kernel_spmd(
        nc,
        in_maps,
        core_ids=list(range(N_CORES)),
        trace=bool(int(os.environ.get("MOE_TRACE", "0"))),
    )
    LAST_RESULT = res
    out = np.concatenate([res.results[c]["out_sh"] for c in range(N_CORES)], axis=0)
    return out.reshape(B, S, H)
